# revision 4
# baseline (speedup 1.0000x reference)
"""Trainium2 Bass kernel for the JaCDE dense-MLP vector-field problem.

v4 = v3b apps structure (two 128-row batch chunks, m-merged [128,256] tiles,
bf16 loop matmuls, DVE gates, GpSimd accumulation, software-pipelined
prologue, one-bank [pA|pB] psum tiles) with the l1 layer computed by hi/lo
bf16 decomposition instead of fp32 matmuls:

  a @ b ~= ah@bh + ah@bl + al@bh   (hi = bf16(v), lo = bf16(v - hi))

which is exact to ~2^-16 — enough to reproduce the reference relu mask
(the mask breaks below ~2^-14; fp32r's tf32-class rounding measurably
fails at 1.2e-1 rel err) — while running at bf16 speed: fp32 matmuls cost
4 cycles/column on the PE, bf16 one.  The hi halves double as the
apps-phase weights, so no on-device bf16 conversion copies are needed,
and total input DMA bytes stay the same as fp32.

Startup (single-shot) structure: input DMA in arrival-ordered slices with
slice-major matmul issue, one activation-table load (dummy sigmoid first,
'sigmoid_and_others' covers Relu+Sigmoid), PE warmup overlapping the DMA,
dtc on DVE in the cold prologue (shortest chain) but on Pool in the
dripped steady-state prologue (keeps DVE for the apps).

Sharding: pure data parallel, batch 2048 -> 8 cores x 256 rows.
"""

import numpy as np
import ml_dtypes

import concourse.tile as tile
from concourse import bacc, mybir
from concourse.bass_utils import run_bass_kernel_spmd

B, H, IN = 2048, 256, 64
K_TERMS = 8
N_CORES = 8
BL = B // N_CORES  # 256 batch rows per core
HH = H // 2  # 128
NCH = 2  # batch chunks per core
CB = BL // NCH  # 128 rows per chunk
CW = 2 * CB  # chunk tile free width (m-merged)

f32 = mybir.dt.float32
bf16 = mybir.dt.bfloat16

_ALU = mybir.AluOpType
_ACT = mybir.ActivationFunctionType
N_WARMUP_MM = 10
DRIP = 3  # prologue matmuls dripped per app slot

# bf16 blob column offsets (all [128, .] on chip; host packs along axis 1).
# Ordered so each DMA slice unlocks the next l1 accumulation group.
_WXH = 0
_WXL = 256
_XH = 512
_XL = 768
_WHH0 = 1024
_WHL0 = 1280
_HH0 = 1536
_HL0 = 1792
_WHH1 = 2048
_WHL1 = 2304
_HH1 = 2560
_HL1 = 2816
_FC = 3072   # b0 halves + (-2*b1) halves, f32 bit-packed as 8 bf16 cols
_WO = 3080   # wout.T halves side by side, bf16   [128, 512]
_XD = 3592   # xdot.T padded, bf16                [128, 256]
FB_COLS = 3848
_S1 = 1024  # [wx | x] group
_S2 = 2048  # k=0 [wh | h] group
_S3 = 3072  # k=1 [wh | h] group
# rest: biases | wout | xdot


def _build(repeat=1, loop=0):
    assert loop % 2 == 0, "device loop must be even (2 repeats per iteration)"
    nc = bacc.Bacc(None, target_bir_lowering=False)

    fb = nc.dram_tensor("fb", [128, FB_COLS], bf16, kind="ExternalInput")
    hdT = nc.dram_tensor("hdT", [H, BL], f32, kind="ExternalOutput")

    with tile.TileContext(nc) as tc:
        with (
            tc.tile_pool(name="wpool", bufs=1) as wpool,
            tc.tile_pool(name="apool", bufs=1) as apool,
            tc.tile_pool(name="rot", bufs=18) as rot,
            tc.tile_pool(name="tgp", bufs=6) as tgp,
            tc.tile_pool(name="ps", bufs=6, space="PSUM") as ps,
            tc.tile_pool(name="psp", bufs=2, space="PSUM") as psp,
        ):
            # ---- input DMAs, critical first ----
            fb_sb = wpool.tile([128, FB_COLS], bf16, tag="fb")
            nc.sync.dma_start(fb_sb[:, 0:_S1], fb[:, 0:_S1])

            # ---- PE warmup memsets early so warmup MMs start ASAP ----
            wu_w = wpool.tile([HH, HH], bf16, tag="wu_w")
            wu_v = wpool.tile([HH, BL], bf16, tag="wu_v")
            nc.vector.memset(wu_w[:].bitcast(f32), 0.0)
            nc.vector.memset(wu_v[:].bitcast(f32), 0.0)

            # ---- act-table preload: dummy sigmoid is the first ACT op, so
            # the single 'sigmoid_and_others' load (contains Relu too)
            # overlaps the input DMA.
            dum = wpool.tile([128, 1], f32, tag="dum")
            nc.vector.memset(dum[:], 0.0)
            nc.scalar.activation(dum[:], dum[:], _ACT.Sigmoid)

            # ---- remaining inputs ----
            nc.sync.dma_start(fb_sb[:, _S1:_S2], fb[:, _S1:_S2])
            nc.sync.dma_start(fb_sb[:, _S2:_S3], fb[:, _S2:_S3])
            nc.sync.dma_start(fb_sb[:, _S3:FB_COLS], fb[:, _S3:FB_COLS])

            wxh = fb_sb[:, _WXH : _WXH + H]
            wxl = fb_sb[:, _WXL : _WXL + H]
            xh = fb_sb[:, _XH : _XH + BL]
            xl = fb_sb[:, _XL : _XL + BL]
            whh = [fb_sb[:, _WHH0 : _WHH0 + H], fb_sb[:, _WHH1 : _WHH1 + H]]
            whl = [fb_sb[:, _WHL0 : _WHL0 + H], fb_sb[:, _WHL1 : _WHL1 + H]]
            hh = [fb_sb[:, _HH0 : _HH0 + BL], fb_sb[:, _HH1 : _HH1 + BL]]
            hl = [fb_sb[:, _HL0 : _HL0 + BL], fb_sb[:, _HL1 : _HL1 + BL]]
            fc_v = fb_sb[:, _FC : _FC + 8].bitcast(f32)  # [128, 4] f32
            b0_sb = fc_v[:, 0:2]
            nb1 = fc_v[:, 2:4]
            woB = [fb_sb[:, _WO : _WO + H], fb_sb[:, _WO + H : _WO + 2 * H]]
            xdB = fb_sb[:, _XD : _XD + BL]
            whB = whh  # hi halves double as the apps-phase weights
            wxB = wxh

            # ---- PE warmup (HAM clock gate), overlaps DMA ----
            if N_WARMUP_MM:
                wu_p = psp.tile([HH, BL], f32, tag="psp")
                for _ in range(N_WARMUP_MM):
                    nc.tensor.matmul(wu_p[:], wu_w[:], wu_v[:], start=True, stop=True)

            # ---- A/B state buffers ----
            relu_mg = [apool.tile([HH, 2 * BL], bf16, tag=f"relu_mg{s}", name=f"relu_mg{s}")
                       for s in range(2)]
            sg_mg = [apool.tile([HH, 2 * BL], f32, tag=f"sg_mg{s}", name=f"sg_mg{s}")
                     for s in range(2)]
            sq_mg = [apool.tile([HH, 2 * BL], f32, tag=f"sq_mg{s}", name=f"sq_mg{s}")
                     for s in range(2)]
            dtc = [[apool.tile([HH, CW], f32, tag=f"dtc{s}{c}", name=f"dtc{s}{c}")
                    for c in range(NCH)] for s in range(2)]
            hdc = [[apool.tile([HH, CW], f32, tag=f"hdc{s}{c}", name=f"hdc{s}{c}")
                    for c in range(NCH)] for s in range(2)]

            # l1 hi/lo term table: (lhsT source, rhs source, partition rows)
            l1_terms = [
                (wxh, xh, IN), (wxl, xh, IN), (wxh, xl, IN),
                (whh[0], hh[0], 128), (whl[0], hh[0], 128), (whh[0], hl[0], 128),
                (whh[1], hh[1], 128), (whl[1], hh[1], 128), (whh[1], hl[1], 128),
            ]

            def prologue_chunks(s, dve_dtc=False):
                """Prologue computing state s, as a list of closures: index 0
                is elementwise-issue; the rest are single PE matmuls to
                spread through the apps phase.  dve_dtc computes dtc with one
                DVE STT per chunk (shortest latency, for the cold standalone
                prologue); otherwise dtc goes through Pool (sq then subtract)
                to keep the dripped steady-state prologue off the DVE."""
                st = {}

                def issue_head():
                    for c in range(NCH):
                        nc.gpsimd.memset(hdc[s][c][:], 0.0)

                def l1_mm(m, t):
                    def go():
                        if t == 0:
                            p = psp.tile([HH, BL], f32, tag="psp", name=f"l1ps{s}{m}")
                            st[("l1", m)] = p
                        else:
                            p = st[("l1", m)]
                        w, v, rows = l1_terms[t]
                        nc.tensor.matmul(
                            p[:], w[0:rows, m * HH : (m + 1) * HH], v[0:rows, :],
                            start=(t == 0), stop=(t == len(l1_terms) - 1))
                        if t == len(l1_terms) - 1:
                            nc.scalar.activation(
                                relu_mg[s][:, m * BL : (m + 1) * BL],
                                p[:], _ACT.Relu, bias=b0_sb[:, m : m + 1])
                    return go

                def lout_mm(m, k):
                    def go():
                        if k == 0:
                            p = psp.tile([HH, BL], f32, tag="psp", name=f"lout{s}{m}")
                            st[("lout", m)] = p
                        else:
                            p = st[("lout", m)]
                        nc.tensor.matmul(
                            p[:], woB[k][:, m * HH : (m + 1) * HH],
                            relu_mg[s][:, k * BL : (k + 1) * BL],
                            start=(k == 0), stop=(k == 1))
                        if k == 1:
                            nc.scalar.activation(
                                sg_mg[s][:, m * BL : (m + 1) * BL],
                                p[:], _ACT.Sigmoid, bias=nb1[:, m : m + 1],
                                scale=-2.0)
                            if dve_dtc:
                                if m == 1:
                                    # dtc[c] = (s-1)*s in one DVE STT/chunk
                                    for c in range(NCH):
                                        src = sg_mg[s][:].rearrange(
                                            "p (m b) -> p m b", m=2
                                        )[:, :, c * CB : (c + 1) * CB]
                                        nc.vector.scalar_tensor_tensor(
                                            dtc[s][c][:].rearrange("p (m b) -> p m b", m=2),
                                            src, -1.0, src,
                                            _ALU.add, _ALU.mult)
                            else:
                                # sq = s*s on ACT (Square is in the loaded
                                # sigmoid_and_others set; keeps Pool free
                                # for the hdc accumulation)
                                nc.scalar.activation(
                                    sq_mg[s][:, m * BL : (m + 1) * BL],
                                    sg_mg[s][:, m * BL : (m + 1) * BL],
                                    _ACT.Square)
                                if m == 1:
                                    # dtc[c] = s^2 - s, strided (m,b) slices
                                    for c in range(NCH):
                                        nc.gpsimd.tensor_tensor(
                                            dtc[s][c][:].rearrange("p (m b) -> p m b", m=2),
                                            sq_mg[s][:].rearrange("p (m b) -> p m b", m=2)[:, :, c * CB : (c + 1) * CB],
                                            sg_mg[s][:].rearrange("p (m b) -> p m b", m=2)[:, :, c * CB : (c + 1) * CB],
                                            _ALU.subtract)
                    return go

                # slice-major issue order: each DMA slice unlocks the next
                # group of matmuls, so the PE never stalls on a later slice
                # while an earlier-ready matmul sits behind it in the queue
                mms = []
                for t in range(len(l1_terms)):
                    mms.append(l1_mm(0, t))
                    mms.append(l1_mm(1, t))
                mms += [lout_mm(0, 0), lout_mm(1, 0), lout_mm(0, 1), lout_mm(1, 1)]
                return issue_head, mms

            def emit_repeat(s, s_next, emit_pro):
                """Apps phase on state s; prologue for state s_next spread in."""
                if emit_pro:
                    pro_head, pro_mms = prologue_chunks(s_next)
                    pro_head()
                else:
                    pro_mms = []
                pro_i = 0

                def drip(n):
                    nonlocal pro_i
                    for _ in range(n):
                        if pro_i < len(pro_mms):
                            pro_mms[pro_i]()
                            pro_i += 1

                vin = [None, None]
                for app in range(K_TERMS + 1):
                    w = [tgp.tile([HH, CW], bf16, tag=f"w{c}", name=f"w{c}") for c in range(NCH)]
                    # one PSUM bank per (app, chunk): [pA | pB] halves, so 6
                    # banks give 3 apps of pipeline depth
                    pt = [ps.tile([HH, 2 * CW], f32, tag="ps", name=f"pt{c}") for c in range(NCH)]
                    pA = [pt[c][:, 0:CW] for c in range(NCH)]
                    pB = [pt[c][:, CW : 2 * CW] for c in range(NCH)]
                    if app == 0:
                        for c in range(NCH):
                            for m in range(2):
                                nc.tensor.matmul(
                                    pA[c][:, m * CB : (m + 1) * CB],
                                    wxB[0:IN, m * HH : (m + 1) * HH],
                                    xdB[0:IN, c * CB : (c + 1) * CB],
                                    start=True, stop=True)
                        for c in range(NCH):
                            nc.vector.scalar_tensor_tensor(
                                w[c][:],
                                relu_mg[s][:].rearrange("p (m b) -> p m b", m=2)[:, :, c * CB : (c + 1) * CB],
                                0.0, pA[c],
                                _ALU.is_gt, _ALU.mult)
                    else:
                        for c in range(NCH):
                            for m in range(2):
                                for k in range(2):
                                    nc.tensor.matmul(
                                        pA[c][:, m * CB : (m + 1) * CB],
                                        whB[k][:, m * HH : (m + 1) * HH],
                                        vin[c][:, k * CB : (k + 1) * CB],
                                        start=(k == 0), stop=(k == 1))
                        for c in range(NCH):
                            nc.vector.scalar_tensor_tensor(
                                w[c][:],
                                relu_mg[s][:].rearrange("p (m b) -> p m b", m=2)[:, :, c * CB : (c + 1) * CB],
                                0.0, pA[c],
                                _ALU.is_gt, _ALU.mult)
                    for c in range(NCH):
                        for m in range(2):
                            for k in range(2):
                                nc.tensor.matmul(
                                    pB[c][:, m * CB : (m + 1) * CB],
                                    woB[k][:, m * HH : (m + 1) * HH],
                                    w[c][:, k * CB : (k + 1) * CB],
                                    start=(k == 0), stop=(k == 1))
                    drip(DRIP if app < K_TERMS else len(pro_mms))
                    vout = [rot.tile([HH, CW], bf16, tag=f"v{c}", name=f"v{c}") for c in range(NCH)]
                    for c in range(NCH):
                        # vout = (-4 * dtc) * pB = dtanh * pB
                        nc.vector.scalar_tensor_tensor(
                            vout[c][:], dtc[s][c][:], -4.0, pB[c],
                            _ALU.mult, _ALU.mult)
                        nc.gpsimd.tensor_add(hdc[s][c][:], hdc[s][c][:], vout[c][:])
                    vin = vout

                for c in range(NCH):
                    # one 3D DMA per chunk: [q, m, b] -> hdT[m*128+q, c*128+b]
                    nc.sync.dma_start(
                        hdT.rearrange("(m q) w -> q m w", m=2)[:, :, c * CB : (c + 1) * CB],
                        hdc[s][c][:].rearrange("p (m b) -> p m b", m=2))

            def emit_prologue_standalone(s):
                pro_head, pro_mms = prologue_chunks(s, dve_dtc=True)
                pro_head()
                for mm in pro_mms:
                    mm()

            emit_prologue_standalone(0)
            if loop:
                with tc.For_i(0, loop // 2, 1):
                    for r in range(repeat):
                        emit_repeat(r % 2, (r + 1) % 2, True)
                    for r in range(repeat):
                        emit_repeat((r + repeat) % 2, (r + repeat + 1) % 2, True)
            else:
                for r in range(repeat):
                    emit_repeat(r % 2, (r + 1) % 2, r < repeat - 1)

    nc.compile()
    return nc


_NC = {}


def _get_nc(repeat=1, loop=0):
    key = (repeat, loop)
    if key not in _NC:
        _NC[key] = _build(repeat, loop)
    return _NC[key]


def _hilo(a):
    """Split f32 array into (hi, lo) bf16 parts: a ~= hi + lo."""
    hi = a.astype(ml_dtypes.bfloat16)
    lo = (a - hi.astype(np.float32)).astype(ml_dtypes.bfloat16)
    return hi, lo


def make_in_maps(h, x, xdot, wx, wh, wout, b0, b1):
    h = np.asarray(h, np.float32)
    x = np.asarray(x, np.float32)
    xdot = np.asarray(xdot, np.float32)
    wx = np.asarray(wx, np.float32)
    wh = np.asarray(wh, np.float32)
    wout = np.asarray(wout, np.float32)
    b0 = np.asarray(b0, np.float32)
    b1 = np.asarray(b1, np.float32)

    def pad128(a):
        out = np.zeros((128, a.shape[1]), a.dtype)
        out[: a.shape[0]] = a
        return out

    wxh, wxl = _hilo(wx.T)          # [64, 256]
    whh, whl = _hilo(wh.T)          # [256, 256]
    woT = wout.T
    fc = np.zeros((128, 4), np.float32)
    fc[:, 0] = b0[:HH]
    fc[:, 1] = b0[HH:]
    fc[:, 2] = -2.0 * b1[:HH]
    fc[:, 3] = -2.0 * b1[HH:]
    fc_b = fc.view(ml_dtypes.bfloat16)  # [128, 8] same bytes
    wo = np.concatenate([woT[0:HH], woT[HH:H]], axis=1).astype(ml_dtypes.bfloat16)

    in_maps = []
    for i in range(N_CORES):
        sl = slice(i * BL, (i + 1) * BL)
        hT = h[sl].T                 # [256, 256]
        hhT, hlT = _hilo(hT)
        xhT, xlT = _hilo(x[sl].T)    # [64, 256]
        xdp = np.zeros((128, BL), ml_dtypes.bfloat16)
        xdp[:IN] = xdot[sl].T.astype(ml_dtypes.bfloat16)
        fbm = np.concatenate(
            [
                pad128(wxh), pad128(wxl), pad128(xhT), pad128(xlT),
                whh[0:HH], whl[0:HH], hhT[0:HH], hlT[0:HH],
                whh[HH:H], whl[HH:H], hhT[HH:H], hlT[HH:H],
                fc_b, wo, xdp,
            ],
            axis=1,
        )
        assert fbm.shape == (128, FB_COLS), fbm.shape
        in_maps.append({"fb": np.ascontiguousarray(fbm)})
    return in_maps


def kernel(h, x, xdot, wx, wh, wout, b0, b1):
    in_maps = make_in_maps(h, x, xdot, wx, wh, wout, b0, b1)
    res = run_bass_kernel_spmd(_get_nc(), in_maps, core_ids=list(range(N_CORES)))
    out = np.empty((B, H), np.float32)
    for i in range(N_CORES):
        out[i * BL : (i + 1) * BL] = res.results[i]["hdT"].T
    return out
